# revision 13
# baseline (speedup 1.0000x reference)
"""Trainium2 Bass kernel for nn_BitNodeTrellis (fp16 channel-blocked layout).

res[b,n,u,i,j] = logsumexp_s( e1[b,n,(u+uhat[b,n])%2,i,s] + e2[b,n,u,s,j] )

Full shapes: e1,e2 [256, 8192, 2, 2, 2] f32, uhat [256, 8192] int32.
Data-parallel over B1=256: each of 8 NeuronCores gets 32 codewords
(ROWS = 262144 independent rows of 8 channels).

Host-side marshalling (not device compute): inputs are cast to fp16 and
repacked channel-blocked — per partition-tile, the 8 channels of ft
consecutive rows are stored as 8 contiguous ft-element runs, and each
tile's a|b|w segments are packed into one DRAM region so a single DMA
feeds a tile. Every DVE instruction reads/writes unit-stride 16-bit
slices (packed 2x tensor_tensor mode). uhat becomes an fp16 0/1 mask;
the XOR-select itself runs on device.

Per tile, on device:
  select (DVE, arithmetic form, in place over a):
      wd = w*(a_hi - a_lo); a_lo += wd; a_hi -= wd      [dif(c^4) = -dif(c)]
  'e' tiles (exp domain):
      EA|EB = exp(asel|b)  [one ACT pass; host pre-shifted a by -SHIFT]
      M1[z] = EA[ca0]*EB[cb0]; M2[z] = EA[ca1]*EB[cb1]  [DVE]
      R = M1 + M2   [PE: identity-weight matmuls accumulated in PSUM]
      out = Ln(R * e^SHIFT)  [ACT, PSUM -> fp16 SBUF]
  'b' tiles (softplus form):
      v0[z] = asel[ca0]+b[cb0]; v1[z] = asel[ca1]+b[cb1]  [DVE]
      d = v1 - v0   [PE: +I.v1 + (-I).v0 in PSUM]
      E = exp(d) (f32), sp2 = Ln(E*e^S + e^S) = softplus(d)+S  [ACT]
      out = v0 + sp2  [DVE]
The e^{x-SHIFT} shift keeps exp-domain intermediates in normal fp16
range for N(0,1)-scale inputs (R <= 2*e^11 < 65504, EA >= e^-8).
"""

import numpy as np

import concourse.bacc as bacc
import concourse.mybir as mybir
import concourse.tile as tile
from concourse.bass_utils import run_bass_kernel_spmd

F16 = mybir.dt.float16
F32 = mybir.dt.float32

P = 128
ACT = mybir.ActivationFunctionType

B1, B2 = 256, 8192
NCORES = 8
B1_SH = B1 // NCORES                  # 32 codewords per core
ROWS = B1_SH * B2                     # 262144 rows per core
RPP = ROWS // P                       # 2048 rows per partition

# (algo, rows-per-partition) per tile; rows must sum to RPP
FTS = [("b", 128), ("e", 512), ("e", 512), ("b", 384), ("b", 256), ("e", 192), ("e", 64)]

SHIFT = 2.0                            # exp-domain range shift
COMBINED_ACT_TABLE = "natural_log_exp_and_others"


class _combined_act_table:
    """Constrain bacc's activation-table chooser to the one real table set
    that contains Exp, Ln and Copy, so it emits a single LoadActFuncSet
    instead of reloading the LUT on every Exp<->Ln alternation."""

    def __enter__(self):
        self._orig = bacc.get_activation_tables
        orig = self._orig

        def constrained(arch):
            tabs = orig(arch)
            need = {ACT.Exp, ACT.Ln, ACT.Copy}
            if not need.issubset(tabs.get(COMBINED_ACT_TABLE, set())):
                return tabs
            return {
                name: (s if name == COMBINED_ACT_TABLE else set())
                for name, s in tabs.items()
            }

        bacc.get_activation_tables = constrained

    def __exit__(self, *a):
        bacc.get_activation_tables = self._orig


def build_program(fts=None, repeat=1, use_pe=True):
    if fts is None:
        fts = FTS
    rpp = sum(ft for _, ft in fts)
    assert rpp * P == ROWS
    ftmax = max(ft for _, ft in fts)

    nc = bacc.Bacc(
        "TRN2",
        target_bir_lowering=False,
        debug=False,
        num_devices=NCORES,
    )
    ESQ = float(np.exp(SHIFT))
    # packed per-tile input: [a(8ft) | b(8ft) | w(ft)] fp16, 17*rpp columns
    pk_d = nc.dram_tensor("pk", [P, rpp * 17], F16, kind="ExternalInput").ap()
    # ident carries [I | -I] fp16 plus a f32 column holding e^SHIFT (Ln bias)
    id_d = nc.dram_tensor("ident", [P, 258], F16, kind="ExternalInput").ap()
    out_d = nc.dram_tensor("out", [P, rpp * 8], F16, kind="ExternalOutput").ap()

    def body(tc):
        with (
            tc.tile_pool(name="stat", bufs=1) as stat,
            tc.tile_pool(name="inp", bufs=4) as inp,
            tc.tile_pool(name="scr", bufs=4) as scr,
            tc.tile_pool(name="scr32", bufs=3) as scr32,
            tc.tile_pool(name="ps", bufs=2, space="PSUM") as psp,
            tc.tile_pool(name="outp", bufs=4) as outp,
        ):
            id_t = stat.tile([P, 258], F16, tag="ident")
            nc.gpsimd.dma_start(id_t[:], id_d)
            ipos = id_t[:, 0:128]
            ineg = id_t[:, 128:256]
            esq_b = id_t[:, 256:258].bitcast(F32)

            f0 = 0
            for algo, ft in fts:
                pk_t = inp.tile([P, ftmax * 17], F16, tag="pk")
                a = pk_t[:, : ft * 8]
                b = pk_t[:, ft * 8 : ft * 16]
                w = pk_t[:, ft * 16 : ft * 17]
                nc.sync.dma_start(
                    pk_t[:, : ft * 17], pk_d[:, f0 * 17 : (f0 + ft) * 17]
                )

                # u-swap select, arithmetic form, in place over a (w is 0/1):
                #   wd = w*(a_hi - a_lo); a_lo += wd; a_hi -= wd
                wd_t = scr.tile([P, ftmax * 4], F16, tag="wd")
                wd = wd_t[:, : ft * 4]
                a_lo = a[:, : ft * 4]
                a_hi = a[:, ft * 4 :]
                wd4 = wd.rearrange("p (c f) -> p c f", c=4)
                wbc = w.unsqueeze(1).broadcast_to([P, 4, ft])
                nc.vector.tensor_sub(wd, a_hi, a_lo)
                nc.vector.tensor_mul(wd4, wd4, wbc)
                nc.vector.tensor_add(a_lo, a_lo, wd)
                nc.vector.tensor_sub(a_hi, a_hi, wd)

                t1_t = scr.tile([P, ftmax * 8], F16, tag="t1")
                t2_t = scr.tile([P, ftmax * 8], F16, tag="t2")
                t1 = t1_t[:, : ft * 8]
                t2 = t2_t[:, : ft * 8]
                r_t = outp.tile([P, ftmax * 8], F16, tag="r")
                r = r_t[:, : ft * 8]

                if algo == "e":
                    # one pass: EA|EB = exp(asel|b); host pre-shifted a
                    nc.scalar.activation(
                        pk_t[:, : ft * 16], pk_t[:, : ft * 16], ACT.Exp
                    )
                    eav = a.rearrange("p (u i s f) -> p u i s f", u=2, i=2, s=2)
                    ebv = b.rearrange("p (u s j f) -> p u s j f", u=2, s=2, j=2)
                    m1v = t1.rearrange("p (u i j f) -> p u i j f", u=2, i=2, j=2)
                    m2v = t2.rearrange("p (u i j f) -> p u i j f", u=2, i=2, j=2)
                    sh5 = [P, 2, 2, 2, ft]
                    nc.vector.tensor_mul(
                        m1v,
                        eav[:, :, :, 0].unsqueeze(3).broadcast_to(sh5),
                        ebv[:, :, 0].unsqueeze(2).broadcast_to(sh5),
                    )
                    nc.vector.tensor_mul(
                        m2v,
                        eav[:, :, :, 1].unsqueeze(3).broadcast_to(sh5),
                        ebv[:, :, 1].unsqueeze(2).broadcast_to(sh5),
                    )
                    if use_pe:
                        for h in range(2):
                            ps_t = psp.tile([P, ftmax * 4], F32, tag="ps")
                            ps = ps_t[:, : ft * 4]
                            base = h * ft * 4
                            for c0 in range(0, ft * 4, 512):
                                c1 = min(c0 + 512, ft * 4)
                                nc.tensor.matmul(
                                    ps[:, c0:c1], ipos,
                                    t1[:, base + c0 : base + c1],
                                    start=True, stop=False,
                                )
                                nc.tensor.matmul(
                                    ps[:, c0:c1], ipos,
                                    t2[:, base + c0 : base + c1],
                                    start=False, stop=True,
                                )
                            nc.scalar.activation(
                                r[:, base : base + ft * 4], ps, ACT.Ln, scale=ESQ
                            )
                    else:
                        nc.vector.tensor_add(t1, t1, t2)
                        nc.scalar.activation(r, t1, ACT.Ln, scale=ESQ)
                else:
                    # v0 in t1, v1 in t2
                    av = a.rearrange("p (u i s f) -> p u i s f", u=2, i=2, s=2)
                    bv = b.rearrange("p (u s j f) -> p u s j f", u=2, s=2, j=2)
                    v0v = t1.rearrange("p (u i j f) -> p u i j f", u=2, i=2, j=2)
                    v1v = t2.rearrange("p (u i j f) -> p u i j f", u=2, i=2, j=2)
                    sh5 = [P, 2, 2, 2, ft]
                    nc.vector.tensor_add(
                        v0v,
                        av[:, :, :, 0].unsqueeze(3).broadcast_to(sh5),
                        bv[:, :, 0].unsqueeze(2).broadcast_to(sh5),
                    )
                    nc.vector.tensor_add(
                        v1v,
                        av[:, :, :, 1].unsqueeze(3).broadcast_to(sh5),
                        bv[:, :, 1].unsqueeze(2).broadcast_to(sh5),
                    )
                    if use_pe:
                        for h in range(2):
                            ps_t = psp.tile([P, ftmax * 4], F32, tag="ps")
                            ps = ps_t[:, : ft * 4]
                            base = h * ft * 4
                            for c0 in range(0, ft * 4, 512):
                                c1 = min(c0 + 512, ft * 4)
                                nc.tensor.matmul(
                                    ps[:, c0:c1], ipos,
                                    t2[:, base + c0 : base + c1],
                                    start=True, stop=False,
                                )
                                nc.tensor.matmul(
                                    ps[:, c0:c1], ineg,
                                    t1[:, base + c0 : base + c1],
                                    start=False, stop=True,
                                )
                            e32_t = scr32.tile([P, ftmax * 4], F32, tag="e32")
                            e32 = e32_t[:, : ft * 4]
                            nc.scalar.activation(e32, ps, ACT.Exp)
                            # sp2 half: Ln(e^d*e^S + e^S) into t2 half
                            nc.scalar.activation(
                                t2[:, base : base + ft * 4], e32,
                                ACT.Ln, bias=esq_b, scale=ESQ,
                            )
                    else:
                        e32_t = scr32.tile([P, ftmax * 8], F32, tag="e32")
                        e32 = e32_t[:, : ft * 8]
                        nc.vector.tensor_sub(t2, t2, t1)
                        nc.scalar.activation(e32, t2, ACT.Exp)
                        nc.scalar.activation(t2, e32, ACT.Ln, bias=esq_b, scale=ESQ)
                    nc.vector.tensor_add(r, t1, t2)

                nc.gpsimd.dma_start(out_d[:, f0 * 8 : (f0 + ft) * 8], r)
                f0 += ft

    with _combined_act_table():
        with tile.TileContext(nc) as tc:
            if repeat == 1:
                body(tc)
            else:
                with tc.For_i(0, repeat, 1):
                    body(tc)
        nc.compile()
    return nc


_NC_CACHE = {}


def _get_nc():
    if "nc" not in _NC_CACHE:
        _NC_CACHE["nc"] = build_program(fts=FTS)
    return _NC_CACHE["nc"]


_IDENT = None


def _ident():
    global _IDENT
    if _IDENT is None:
        eye = np.eye(P, dtype=np.float16)
        esq = np.full((P, 1), np.exp(SHIFT), dtype=np.float32)
        _IDENT = np.ascontiguousarray(
            np.concatenate([eye, -eye, esq.view(np.float16)], axis=1)
        )
    return _IDENT


def make_in_maps(e1, e2, uhat, fts=None):
    if fts is None:
        fts = FTS
    e1 = (np.asarray(e1, dtype=np.float32) - SHIFT).astype(np.float16)
    e2 = np.asarray(e2, dtype=np.float32).astype(np.float16)
    wmask = (np.asarray(uhat, dtype=np.int32) == 1).astype(np.float16)
    in_maps = []
    for c in range(NCORES):
        sl = slice(c * B1_SH, (c + 1) * B1_SH)
        ar = e1[sl].reshape(P, RPP, 8)
        br = e2[sl].reshape(P, RPP, 8)
        wr = wmask[sl].reshape(P, RPP)
        parts = []
        f0 = 0
        for _, ft in fts:
            aseg = ar[:, f0 : f0 + ft, :].transpose(0, 2, 1).reshape(P, 8 * ft)
            bseg = br[:, f0 : f0 + ft, :].transpose(0, 2, 1).reshape(P, 8 * ft)
            wseg = wr[:, f0 : f0 + ft]
            parts.extend([aseg, bseg, wseg])
            f0 += ft
        in_maps.append(
            {
                "pk": np.ascontiguousarray(np.concatenate(parts, axis=1)),
                "ident": _ident(),
            }
        )
    return in_maps


def _unpack_out(res_core, fts):
    """[P, 8*RPP] fp16 channel-blocked -> [ROWS, 8] f32."""
    parts = []
    f0 = 0
    for _, ft in fts:
        seg = res_core[:, f0 * 8 : (f0 + ft) * 8].reshape(P, 8, ft)
        parts.append(seg.transpose(0, 2, 1))
        f0 += ft
    full = np.concatenate(parts, axis=1)  # [P, RPP, 8]
    return full.reshape(ROWS, 8).astype(np.float32)


def kernel(e1: np.ndarray, e2: np.ndarray, uhat: np.ndarray) -> np.ndarray:
    nc = _get_nc()
    in_maps = make_in_maps(e1, e2, uhat)
    res = run_bass_kernel_spmd(nc, in_maps, list(range(NCORES)))
    out = np.empty((B1, B2, 2, 2, 2), dtype=np.float32)
    for c in range(NCORES):
        out[c * B1_SH : (c + 1) * B1_SH] = _unpack_out(
            res.results[c]["out"], FTS
        ).reshape(B1_SH, B2, 2, 2, 2)
    return out
